# revision 35
# baseline (speedup 1.0000x reference)
"""Trainium2 Bass kernel for MultiHeadDilatedAttention.

Full inputs in, full output out. Sharding: 8 cores = (batch b in 0..3) x
(segment-position half). Each (b, s) pair is an independent attention problem
(attention runs across segments n at fixed position-in-segment s), so each
core handles b = c//2 and 64 of the 128 s values. No collectives needed: the
output rows t = s*64 + dil*l for a core's s-range form a contiguous chunk of
y[b].

Per-core dataflow (all matmuls bf16 with fp32 PSUM accumulation):
  x (fp32, cast-DMA to bf16) -> PE-transpose to x^T [e, rows]
  Q^T/K^T/V^T = W^T-chunks (stationary) x x^T         [dk, (l,s)]
  V^T -> PE-transpose -> V natural [n(=l) slots, dv]
  KQ[n,m] per s -> +mask -> exp(NORM*.) -> row-normalize over m -> smKQ
  att^T[v,m] = V_s^T-matmul(smKQ)                      [dv, (l,s)]
  y rows grouped by offset-class: y[(o,s), e] = sum_h att^T-chunk^T @ W_out_h^T + b
"""

import os
from contextlib import ExitStack

import numpy as np
import ml_dtypes

import concourse.bass as bass
import concourse.mybir as mybir
import concourse.tile as tile
from concourse import bacc
from concourse.masks import make_identity
from concourse.bass_utils import run_bass_kernel_spmd

F32 = mybir.dt.float32
BF16 = mybir.dt.bfloat16
AX = mybir.AxisListType

B, T, E = 4, 8192, 1024
SEG = 128          # segment size (= #s positions overall)
NB = T // SEG      # 64 segments
NS = 64            # s values per core
ROWS = NB * NS     # 4096 rows per core
DK = 128
H = 4
DILS = [1, 2, 4, 8]
LS = [NB // d for d in DILS]       # [64, 32, 16, 8]
SLOTL = [max(l, 32) for l in LS]   # partition slot stride: [64, 32, 32, 32]
G = [128 // sl for sl in SLOTL]    # s-slots per 128 partitions: [2, 4, 4, 4]
MOFF = [0, 64, 96, 112]            # mask column offsets, widths LS
NORM = float(1.0 / np.sqrt(DK))
NEG = -1.0e10
NECHUNK = E // 128                 # 8

# out-projection classes: offsets o in [0,64) grouped by which heads hit them.
# class entries: (o_list, heads). o = dil_h * l must have dil_h | o.
def _classes():
    out = []
    o_all = list(range(64))
    out.append(([o for o in o_all if o % 2 == 1], [0]))           # odd
    out.append(([o for o in o_all if o % 4 == 2], [0, 1]))        # 2 mod 4
    out.append(([o for o in o_all if o % 8 == 4], [0, 1, 2]))     # 4 mod 8
    out.append(([o for o in o_all if o % 8 == 0], [0, 1, 2, 3]))  # 0 mod 8
    return out


CLASSES = _classes()
# atT[h] column layout: per-class blocks (so out-proj lhsT slices are
# contiguous). HEAD_BLOCKS[h] = [(cid, l_list)], HEAD_OFF[h][cid] = col offset.
HEAD_BLOCKS = {}
HEAD_OFF = {}
for _h in range(H):
    blocks, offs, off = [], {}, 0
    for _cid, (_ol, _heads) in enumerate(CLASSES):
        if _h in _heads:
            ll = [o // DILS[_h] for o in _ol]
            blocks.append((_cid, ll))
            offs[_cid] = off
            off += len(ll) * NS
    HEAD_BLOCKS[_h] = blocks
    HEAD_OFF[_h] = offs
    assert off == LS[_h] * NS


def build_program(max_phase: int = 5, sub: int = 9, dheads=(0, 1, 2, 3)) -> bass.Bass:
    nc = bacc.Bacc("TRN2", target_bir_lowering=False, debug=False)
    xs = nc.dram_tensor("xs", [ROWS, E], F32, kind="ExternalInput").ap()
    wqkv = nc.dram_tensor("wqkv", [128, 12 * NECHUNK * 128], BF16,
                          kind="ExternalInput").ap()
    wout = nc.dram_tensor("wout", [128, H * E], BF16, kind="ExternalInput").ap()
    maskd = nc.dram_tensor("masks", [128, 120], F32, kind="ExternalInput").ap()
    biasd = nc.dram_tensor("bias", [1, E], BF16, kind="ExternalInput").ap()
    y = nc.dram_tensor("y", [ROWS, E], F32, kind="ExternalOutput").ap()

    try:
        _build_phases(nc, max_phase, xs, wqkv, wout, maskd, biasd, y, sub, dheads)
    except _StopBuild:
        pass
    nc.finalize()
    return nc


class _StopBuild(Exception):
    pass


def _build_phases(nc, max_phase, xs, wqkv, wout, maskd, biasd, y, sub=9, dheads=(0, 1, 2, 3)):
    with ExitStack() as ctx:
        tc = ctx.enter_context(tile.TileContext(nc))

        persist = ctx.enter_context(tc.tile_pool(name="persist", bufs=1))
        ident = persist.tile([128, 128], BF16, tag="ident")
        make_identity(nc, ident)
        ident32 = persist.tile([128, 128], F32, tag="ident32")
        make_identity(nc, ident32)
        ones_t = persist.tile([1, 128], BF16, tag="ones")
        nc.vector.memset(ones_t, 1.0)
        w_sb = persist.tile([128, 12 * NECHUNK * 128], BF16, tag="w_sb")
        nc.sync.dma_start(out=w_sb, in_=wqkv)
        wout_sb = persist.tile([128, H * E], BF16, tag="wout_sb")
        nc.sync.dma_start(out=wout_sb, in_=wout)
        mask_sb = persist.tile([128, 120], F32, tag="mask_sb")
        nc.sync.dma_start(out=mask_sb, in_=maskd)
        bias_sb = persist.tile([1, E], BF16, tag="bias_sb")
        nc.sync.dma_start(out=bias_sb, in_=biasd)

        # persistent per-head tensors
        qkvpool = ctx.enter_context(tc.tile_pool(name="qkv", bufs=1))
        qkv_sb = {}
        for h in range(H):
            for p in range(3):
                qkv_sb[(h, p)] = qkvpool.tile([128, LS[h] * NS], BF16,
                                              tag=f"qkv{h}{p}", name=f"qkv{h}{p}")
        atpool = ctx.enter_context(tc.tile_pool(name="atT", bufs=1))
        atT = [atpool.tile([128, LS[h] * NS], BF16, tag=f"atT{h}", name=f"atT{h}")
               for h in range(H)]
        vnpool = ctx.enter_context(tc.tile_pool(name="vnat", bufs=1))
        # vnat[h]: [128, ngroups, 128]; group gi holds V_s for s = gi*G[h]..+G[h]
        # at partition slots (s % G[h]) * SLOTL[h]
        NGRP = [NS // g for g in G]    # [32, 16, 16, 16]
        vnat = [vnpool.tile([128, NGRP[h], 128], BF16, tag=f"vnat{h}", name=f"vnat{h}")
                for h in range(H)]

        # ---------------- phase A+B: load x, transpose, project QKV --------
        with ExitStack() as pctx:
            xt_pool = pctx.enter_context(tc.tile_pool(name="xt", bufs=1))
            xt = [xt_pool.tile([128, ROWS], BF16, tag=f"xt{ec}", name=f"xt{ec}")
                  for ec in range(NECHUNK)]
            ld_pool = pctx.enter_context(tc.tile_pool(name="xn", bufs=3))
            tp_ps = pctx.enter_context(
                tc.tile_pool(name="tp_ps", bufs=4, space="PSUM"))
            for rt in range(ROWS // 128):
                xn = ld_pool.tile([128, E], F32)
                nc.sync.dma_start(out=xn, in_=xs[rt * 128:(rt + 1) * 128, :])
                for ec in range(NECHUNK):
                    pt = tp_ps.tile([128, 128], F32)
                    nc.tensor.transpose(pt, xn[:, ec * 128:(ec + 1) * 128],
                                        ident32)
                    # copy also casts fp32 -> bf16
                    nc.any.tensor_copy(out=xt[ec][:, rt * 128:(rt + 1) * 128],
                                       in_=pt)

            if max_phase < 2:
                return
            qk_ps = pctx.enter_context(
                tc.tile_pool(name="qk_ps", bufs=4, space="PSUM"))
            for h in range(H):
                L, dil = LS[h], DILS[h]
                ncols = L * NS
                for p in range(3):
                    dst = qkv_sb[(h, p)]
                    for nt in range(ncols // 512):
                        ps = qk_ps.tile([128, 512], F32)
                        l0 = nt * (512 // NS)
                        for ec in range(NECHUNK):
                            wi = ((h * 3 + p) * NECHUNK + ec) * 128
                            lhsT = w_sb[:, wi:wi + 128]
                            rhs = xt[ec].rearrange(
                                "p (l j s) -> p l j s", j=dil, s=NS
                            )[:, l0:l0 + 8, 0, :]
                            nc.tensor.matmul(ps, lhsT, rhs,
                                             start=(ec == 0),
                                             stop=(ec == NECHUNK - 1))
                        if p == 2:
                            # V^T stored s-major (col = s*L + l) so later
                            # PE transposes read contiguous column groups
                            out_ap = dst.rearrange(
                                "p (s l) -> p l s", l=L)[:, l0:l0 + 8, :]
                            in_ap = ps.rearrange("p (l s) -> p l s", s=NS)
                            nc.any.tensor_copy(out=out_ap, in_=in_ap)
                        else:
                            nc.any.tensor_copy(
                                out=dst[:, nt * 512:(nt + 1) * 512], in_=ps)

        # ---------------- phase C: V^T -> V natural ------------------------
        if max_phase < 3:
            return
        with ExitStack() as pctx:
            vt_ps = pctx.enter_context(
                tc.tile_pool(name="vt_ps", bufs=4, space="PSUM"))
            for h in range(H):
                L, g = LS[h], G[h]
                vt = qkv_sb[(h, 2)]   # s-major: col = s*L + l
                for gi in range(NGRP[h]):
                    pt = vt_ps.tile([128, 128], BF16)
                    if L >= 32:
                        # g s-values side by side -> out partitions
                        # (s_local * L + l)
                        c0 = gi * g * L
                        nc.tensor.transpose(
                            pt, vt[:, c0:c0 + g * L], ident)
                        nc.any.tensor_copy(out=vnat[h][:, gi, :], in_=pt)
                    else:
                        for k in range(g):
                            s = gi * g + k
                            nc.tensor.transpose(
                                pt[k * 32:k * 32 + L, :],
                                vt[:, s * L:(s + 1) * L], ident,
                                tile_position=(0, k * 32))
                        for k in range(g):
                            nc.any.tensor_copy(
                                out=vnat[h][k * 32:k * 32 + L, gi, :],
                                in_=pt[k * 32:k * 32 + L, :])

        # ---------------- phase D: attention per head ----------------------
        # s per KQ tile and per att sub-batch
        MG = [8, 16, 16, 16]          # m-groups (free) per KQ psum tile
        if max_phase < 4:
            return
        with ExitStack() as pctx:
            kq_ps = pctx.enter_context(
                tc.tile_pool(name="kq_ps", bufs=2, space="PSUM"))
            at_ps = pctx.enter_context(
                tc.tile_pool(name="at_ps", bufs=2, space="PSUM"))
            sm_pool = pctx.enter_context(tc.tile_pool(name="sm", bufs=3))
            small = pctx.enter_context(tc.tile_pool(name="small", bufs=4))
            for h in range(H):
                if h not in dheads:
                    continue
                L, g, sl, mg = LS[h], G[h], SLOTL[h], MG[h]
                s_per = g * mg
                kt_r = qkv_sb[(h, 1)].rearrange("p (l s) -> p l s", s=NS)
                qt_r = qkv_sb[(h, 0)].rearrange("p (l s) -> p l s", s=NS)
                m_sl = mask_sb[:, MOFF[h]:MOFF[h] + L]
                # partition ranges actually written by KQ matmuls (for L<32
                # the top of each 32-slot is unwritten PSUM -> skip it)
                pranges = [(0, 128)] if L >= 32 else \
                    [(k * 32, L) for k in range(g)]
                for S0 in range(0, NS, s_per):
                    ps_kq = kq_ps.tile([128, mg * L], F32, tag="kq")
                    for ci in range(mg):
                        for pi in range(g):
                            s = S0 + ci * g + pi
                            nc.tensor.matmul(
                                ps_kq[pi * sl:pi * sl + L,
                                      ci * L:(ci + 1) * L],
                                kt_r[:, :, s], qt_r[:, :, s],
                                start=True, stop=True,
                                tile_position=(0, pi * sl))
                    if sub < 2:
                        continue
                    numer = sm_pool.tile([128, mg * L], F32, tag="numer")
                    enumer = sm_pool.tile([128, mg * L], BF16, tag="enumer")
                    sums = small.tile([128, mg], F32, tag="sums")
                    recip = small.tile([128, mg], F32, tag="recip")
                    smkq = sm_pool.tile([128, mg * L], BF16, tag="smkq")
                    for p0, pl in pranges:
                        pz = slice(p0, p0 + pl)
                        mk = m_sl[pz, :]
                        mask_bc = bass.AP(tensor=mk.tensor, offset=mk.offset,
                                          ap=[mk.ap[0], [0, mg], mk.ap[1]])
                        nc.vector.tensor_add(numer[pz, :], ps_kq[pz, :],
                                             mask_bc)
                        nc.scalar.activation(
                            enumer[pz, :], numer[pz, :],
                            mybir.ActivationFunctionType.Exp, scale=NORM)
                        nc.vector.reduce_sum(
                            sums[pz, :],
                            enumer[pz, :].rearrange("p (c l) -> p c l", l=L),
                            axis=AX.X)
                        nc.vector.reciprocal(recip[pz, :], sums[pz, :])
                        rc = recip[pz, :]
                        rc_bc = bass.AP(tensor=rc.tensor, offset=rc.offset,
                                        ap=[rc.ap[0], [1, mg], [0, L]])
                        nc.vector.tensor_mul(smkq[pz, :], enumer[pz, :],
                                             rc_bc)
                    # att: one PSUM tile per partition slot — concurrent
                    # row-tiled matmuls must hit different PSUM banks
                    if sub < 3:
                        continue
                    cnt = s_per // g    # s values per slot in this S0 batch
                    for pi in range(g):
                        slot = pi * sl
                        ps_at = at_ps.tile([128, cnt * L], F32,
                                           tag=f"at{pi}", name=f"at{pi}",
                                           bufs=1)
                        for ci in range(cnt):
                            s = S0 + ci * g + pi
                            lhsT = vnat[h][slot:slot + L, s // g, :]
                            rhs = smkq.rearrange(
                                "p (c l) -> p c l", l=L)[slot:slot + L, ci, :]
                            nc.tensor.matmul(ps_at[:, ci * L:(ci + 1) * L],
                                             lhsT, rhs,
                                             start=True, stop=True,
                                             tile_position=(slot, 0))
                        # scatter-copy into atT[h], class-blocked columns
                        if sub < 4:
                            continue
                        in_r = ps_at.rearrange("p (s l) -> p s l", l=L)
                        for cid, ll in HEAD_BLOCKS[h]:
                            nl = len(ll)
                            lst = ll[1] - ll[0]
                            in_ap = in_r[:, :,
                                         ll[0]:ll[-1] + 1:lst] if nl > 1 \
                                else in_r[:, :, ll[0]:ll[0] + 1]
                            off = HEAD_OFF[h][cid]
                            s0 = S0 + pi
                            out_ap = atT[h][:, off:off + nl * NS].rearrange(
                                "p (r s) -> p s r", s=NS
                            )[:, s0:s0 + (cnt - 1) * g + 1:g, :] if g > 1 \
                                else atT[h][:, off:off + nl * NS].rearrange(
                                "p (r s) -> p s r", s=NS)[:, s0:s0 + cnt, :]
                            nc.any.tensor_copy(out=out_ap, in_=in_ap)

        # ---------------- phase E: output projection + store ---------------
        if max_phase < 5:
            return
        with ExitStack() as pctx:
            y_ps = pctx.enter_context(
                tc.tile_pool(name="y_ps", bufs=2, space="PSUM"))
            yo_pool = pctx.enter_context(tc.tile_pool(name="y_sb", bufs=3))
            yv = y.rearrange("(s o) e -> o s e", o=64)
            for cid, (o_list, heads) in enumerate(CLASSES):
                for ci in range(len(o_list) // 2):
                    oa, ob = o_list[2 * ci], o_list[2 * ci + 1]
                    ps_y = y_ps.tile([128, E], F32, tag="y")
                    for half in range(2):
                        cs = half * 512
                        for j, h in enumerate(heads):
                            off = HEAD_OFF[h][cid] + ci * 128
                            lhsT = atT[h][:, off:off + 128]
                            nc.tensor.matmul(
                                ps_y[:, cs:cs + 512], lhsT,
                                wout_sb[:, h * E + cs:h * E + cs + 512],
                                start=(j == 0), stop=False)
                        nc.tensor.matmul(
                            ps_y[:, cs:cs + 512], ones_t,
                            bias_sb[:, cs:cs + 512],
                            start=False, stop=True)
                    y_sb = yo_pool.tile([128, E], F32)
                    nc.any.tensor_copy(out=y_sb, in_=ps_y)
                    out_ap = yv[oa:ob + 1:(ob - oa), :, :]
                    nc.sync.dma_start(out=out_ap, in_=y_sb)
    nc.finalize()
    return nc


_NC = None


def _get_program():
    global _NC
    if _NC is None:
        _NC = build_program()
    return _NC


def _host_inputs(Wk, Wq, Wv, W_out, b_out):
    bf = ml_dtypes.bfloat16
    Wstack = np.stack([Wq, Wk, Wv], 1)                     # [H, 3, 128, 1024]
    tmp = Wstack.reshape(H, 3, 128, NECHUNK, 128)          # [h, p, c, ec, r]
    wqkv_sb = np.ascontiguousarray(
        tmp.transpose(4, 0, 1, 3, 2)).reshape(128, -1).astype(bf)
    wout_sb = np.ascontiguousarray(
        W_out.reshape(E, H, 128).transpose(2, 1, 0)).reshape(128, H * E
                                                             ).astype(bf)
    mask_host = np.full((128, 120), NEG, np.float32)
    for h in range(H):
        L, sl = LS[h], SLOTL[h]
        for p in range(128):
            n = p % sl
            if n < L:
                mask_host[p, MOFF[h]:MOFF[h] + n + 1] = 0.0
            else:
                mask_host[p, MOFF[h]] = 0.0   # keep garbage rows finite
    bias_sb = np.asarray(b_out, np.float32).reshape(1, E).astype(bf)
    return wqkv_sb, wout_sb, mask_host, bias_sb


def kernel(x, Wk, Wq, Wv, W_out, b_out):
    x = np.asarray(x, np.float32)
    wqkv_sb, wout_sb, mask_host, bias_sb = _host_inputs(
        np.asarray(Wk, np.float32), np.asarray(Wq, np.float32),
        np.asarray(Wv, np.float32), np.asarray(W_out, np.float32),
        np.asarray(b_out, np.float32))
    in_maps = []
    for c in range(8):
        b, half = c // 2, c % 2
        xs = np.ascontiguousarray(
            x[b].reshape(NB, SEG, E)[:, half * NS:(half + 1) * NS, :]
        ).reshape(ROWS, E)
        in_maps.append({"xs": xs, "wqkv": wqkv_sb, "wout": wout_sb,
                        "masks": mask_host, "bias": bias_sb})
    nc = _get_program()
    res = run_bass_kernel_spmd(nc, in_maps, core_ids=list(range(8)))
    y = np.empty((B, T, E), np.float32)
    for c in range(8):
        b, half = c // 2, c % 2
        y[b, half * ROWS:(half + 1) * ROWS, :] = res.results[c]["y"]
    return y


# revision 38
# speedup vs baseline: 1.6402x; 1.6402x over previous
"""Trainium2 Bass kernel for MultiHeadDilatedAttention.

Full inputs in, full output out. Sharding: 8 cores = (batch b in 0..3) x
(segment-position half). Each (b, s) pair is an independent attention problem
(attention runs across segments n at fixed position-in-segment s), so each
core handles b = c//2 and 64 of the 128 s values. No collectives needed: the
output rows t = s*64 + dil*l for a core's s-range form a contiguous chunk of
y[b].

Per-core dataflow (all matmuls bf16 with fp32 PSUM accumulation):
  x (fp32, cast-DMA to bf16) -> PE-transpose to x^T [e, rows]
  Q^T/K^T/V^T = W^T-chunks (stationary) x x^T         [dk, (l,s)]
  V^T -> PE-transpose -> V natural [n(=l) slots, dv]
  KQ[n,m] per s -> +mask -> exp(NORM*.) -> row-normalize over m -> smKQ
  att^T[v,m] = V_s^T-matmul(smKQ)                      [dv, (l,s)]
  y rows grouped by offset-class: y[(o,s), e] = sum_h att^T-chunk^T @ W_out_h^T + b
"""

import os
from contextlib import ExitStack

import numpy as np
import ml_dtypes

import concourse.bass as bass
import concourse.mybir as mybir
import concourse.tile as tile
from concourse import bacc
from concourse.masks import make_identity
from concourse.bass_utils import run_bass_kernel_spmd

F32 = mybir.dt.float32
BF16 = mybir.dt.bfloat16
AX = mybir.AxisListType

B, T, E = 4, 8192, 1024
SEG = 128          # segment size (= #s positions overall)
NB = T // SEG      # 64 segments
NS = 64            # s values per core
ROWS = NB * NS     # 4096 rows per core
DK = 128
H = 4
DILS = [1, 2, 4, 8]
LS = [NB // d for d in DILS]       # [64, 32, 16, 8]
SLOTL = [max(l, 32) for l in LS]   # partition slot stride: [64, 32, 32, 32]
G = [128 // sl for sl in SLOTL]    # s-slots per 128 partitions: [2, 4, 4, 4]
MOFF = [0, 64, 96, 112]            # mask column offsets, widths LS
NORM = float(1.0 / np.sqrt(DK))
NEG = -1.0e10
NECHUNK = E // 128                 # 8

# out-projection classes: offsets o in [0,64) grouped by which heads hit them.
# class entries: (o_list, heads). o = dil_h * l must have dil_h | o.
def _classes():
    out = []
    o_all = list(range(64))
    out.append(([o for o in o_all if o % 2 == 1], [0]))           # odd
    out.append(([o for o in o_all if o % 4 == 2], [0, 1]))        # 2 mod 4
    out.append(([o for o in o_all if o % 8 == 4], [0, 1, 2]))     # 4 mod 8
    out.append(([o for o in o_all if o % 8 == 0], [0, 1, 2, 3]))  # 0 mod 8
    return out


CLASSES = _classes()
# atT[h] column layout: per-class blocks (so out-proj lhsT slices are
# contiguous). HEAD_BLOCKS[h] = [(cid, l_list)], HEAD_OFF[h][cid] = col offset.
HEAD_BLOCKS = {}
HEAD_OFF = {}
for _h in range(H):
    blocks, offs, off = [], {}, 0
    for _cid, (_ol, _heads) in enumerate(CLASSES):
        if _h in _heads:
            ll = [o // DILS[_h] for o in _ol]
            blocks.append((_cid, ll))
            offs[_cid] = off
            off += len(ll) * NS
    HEAD_BLOCKS[_h] = blocks
    HEAD_OFF[_h] = offs
    assert off == LS[_h] * NS


def build_program(max_phase: int = 5, sub: int = 9, dheads=(0, 1, 2, 3)) -> bass.Bass:
    nc = bacc.Bacc("TRN2", target_bir_lowering=False, debug=False)
    xs = nc.dram_tensor("xs", [ROWS, E], F32, kind="ExternalInput").ap()
    wqkv = nc.dram_tensor("wqkv", [128, 12 * NECHUNK * 128], BF16,
                          kind="ExternalInput").ap()
    wout = nc.dram_tensor("wout", [128, H * E], BF16, kind="ExternalInput").ap()
    maskd = nc.dram_tensor("masks", [128, 120], F32, kind="ExternalInput").ap()
    biasd = nc.dram_tensor("bias", [128, E], F32, kind="ExternalInput").ap()
    y = nc.dram_tensor("y", [ROWS, E], F32, kind="ExternalOutput").ap()

    try:
        _build_phases(nc, max_phase, xs, wqkv, wout, maskd, biasd, y, sub, dheads)
    except _StopBuild:
        pass
    nc.finalize()
    return nc


class _StopBuild(Exception):
    pass


def _build_phases(nc, max_phase, xs, wqkv, wout, maskd, biasd, y, sub=9, dheads=(0, 1, 2, 3)):
    with ExitStack() as ctx:
        tc = ctx.enter_context(tile.TileContext(nc))

        persist = ctx.enter_context(tc.tile_pool(name="persist", bufs=1))
        ident = persist.tile([128, 128], BF16, tag="ident")
        make_identity(nc, ident)
        w_sb = persist.tile([128, 12 * NECHUNK * 128], BF16, tag="w_sb")
        nc.sync.dma_start(out=w_sb, in_=wqkv)
        wout_sb = persist.tile([128, H * E], BF16, tag="wout_sb")
        nc.sync.dma_start(out=wout_sb, in_=wout)
        mask_sb = persist.tile([128, 120], F32, tag="mask_sb")
        nc.sync.dma_start(out=mask_sb, in_=maskd)
        bias_sb = persist.tile([128, E], F32, tag="bias_sb")
        nc.sync.dma_start(out=bias_sb, in_=biasd)

        # persistent per-head tensors
        qkvpool = ctx.enter_context(tc.tile_pool(name="qkv", bufs=1))
        qkv_sb = {}
        for h in range(H):
            for p in range(3):
                qkv_sb[(h, p)] = qkvpool.tile([128, LS[h] * NS], BF16,
                                              tag=f"qkv{h}{p}", name=f"qkv{h}{p}")
        # ---------------- phase A+B: load x, transpose, project QKV --------
        with ExitStack() as pctx:
            xt_pool = pctx.enter_context(tc.tile_pool(name="xt", bufs=1))
            xt = [xt_pool.tile([128, ROWS], BF16, tag=f"xt{ec}", name=f"xt{ec}")
                  for ec in range(NECHUNK)]
            ld_pool = pctx.enter_context(tc.tile_pool(name="xn", bufs=3))
            tp_ps = pctx.enter_context(
                tc.tile_pool(name="tp_ps", bufs=4, space="PSUM"))
            for rt in range(ROWS // 128):
                xn = ld_pool.tile([128, E], F32, bufs=2)
                nc.sync.dma_start(out=xn, in_=xs[rt * 128:(rt + 1) * 128, :])
                xnb = ld_pool.tile([128, E], BF16, tag="xnb", bufs=2)
                nc.scalar.copy(out=xnb, in_=xn)   # cast fp32 -> bf16 on ACT
                for ec in range(NECHUNK):
                    pt = tp_ps.tile([128, 128], BF16)
                    nc.tensor.transpose(pt, xnb[:, ec * 128:(ec + 1) * 128],
                                        ident)
                    nc.vector.tensor_copy(
                        out=xt[ec][:, rt * 128:(rt + 1) * 128], in_=pt)

            if max_phase < 2:
                return
            qk_ps = pctx.enter_context(
                tc.tile_pool(name="qk_ps", bufs=4, space="PSUM"))
            for h in range(H):
                L, dil = LS[h], DILS[h]
                ncols = L * NS
                for p in range(3):
                    dst = qkv_sb[(h, p)]
                    for nt in range(ncols // 512):
                        ps = qk_ps.tile([128, 512], F32)
                        l0 = nt * (512 // NS)
                        for ec in range(NECHUNK):
                            wi = ((h * 3 + p) * NECHUNK + ec) * 128
                            lhsT = w_sb[:, wi:wi + 128]
                            rhs = xt[ec].rearrange(
                                "p (l j s) -> p l j s", j=dil, s=NS
                            )[:, l0:l0 + 8, 0, :]
                            nc.tensor.matmul(ps, lhsT, rhs,
                                             start=(ec == 0),
                                             stop=(ec == NECHUNK - 1))
                        if p == 2:
                            # V^T stored s-major (col = s*L + l) so later
                            # PE transposes read contiguous column groups
                            out_ap = dst.rearrange(
                                "p (s l) -> p l s", l=L)[:, l0:l0 + 8, :]
                            in_ap = ps.rearrange("p (l s) -> p l s", s=NS)
                            nc.vector.tensor_copy(out=out_ap, in_=in_ap)
                        else:
                            nc.vector.tensor_copy(
                                out=dst[:, nt * 512:(nt + 1) * 512], in_=ps)

        # ---------------- phase C: V^T -> V natural ------------------------
        if max_phase < 3:
            return
        atpool = ctx.enter_context(tc.tile_pool(name="atT", bufs=1))
        # dense layout: col = s*64 + o (zero where dil does not divide o)
        atT = [atpool.tile([128, NS * 64], BF16, tag=f"atT{h}", name=f"atT{h}")
               for h in range(H)]
        for h in range(H):
            if DILS[h] > 1:
                nc.gpsimd.memset(atT[h], 0.0)
        vnpool = ctx.enter_context(tc.tile_pool(name="vnat", bufs=1))
        # vnat[h]: [128, ngroups, 128]; group gi holds V_s for s = gi*G[h]..+G[h]
        # at partition slots (s % G[h]) * SLOTL[h]
        NGRP = [NS // g for g in G]    # [32, 16, 16, 16]
        vnat = [vnpool.tile([128, NGRP[h], 128], BF16, tag=f"vnat{h}", name=f"vnat{h}")
                for h in range(H)]
        with ExitStack() as pctx:
            vt_ps = pctx.enter_context(
                tc.tile_pool(name="vt_ps", bufs=4, space="PSUM"))
            for h in range(H):
                L, g = LS[h], G[h]
                vt = qkv_sb[(h, 2)]   # s-major: col = s*L + l
                for gi in range(NGRP[h]):
                    pt = vt_ps.tile([128, 128], BF16)
                    if L >= 32:
                        # g s-values side by side -> out partitions
                        # (s_local * L + l)
                        c0 = gi * g * L
                        nc.tensor.transpose(
                            pt, vt[:, c0:c0 + g * L], ident)
                        nc.scalar.copy(out=vnat[h][:, gi, :], in_=pt)
                    else:
                        for k in range(g):
                            s = gi * g + k
                            nc.tensor.transpose(
                                pt[k * 32:k * 32 + L, :],
                                vt[:, s * L:(s + 1) * L], ident,
                                tile_position=(0, k * 32))
                        for k in range(g):
                            nc.scalar.copy(
                                out=vnat[h][k * 32:k * 32 + L, gi, :],
                                in_=pt[k * 32:k * 32 + L, :])

        # ---------------- phase D: attention per head ----------------------
        # s per KQ tile and per att sub-batch
        MG = [8, 16, 16, 16]          # m-groups (free) per KQ psum tile
        if max_phase < 4:
            return
        with ExitStack() as pctx:
            kq_ps = pctx.enter_context(
                tc.tile_pool(name="kq_ps", bufs=2, space="PSUM"))
            at_ps = pctx.enter_context(
                tc.tile_pool(name="at_ps", bufs=2, space="PSUM"))
            sm_pool = pctx.enter_context(tc.tile_pool(name="sm", bufs=3))
            small = pctx.enter_context(tc.tile_pool(name="small", bufs=4))
            for h in range(H):
                if h not in dheads:
                    continue
                L, g, sl, mg = LS[h], G[h], SLOTL[h], MG[h]
                s_per = g * mg
                kt_r = qkv_sb[(h, 1)].rearrange("p (l s) -> p l s", s=NS)
                qt_r = qkv_sb[(h, 0)].rearrange("p (l s) -> p l s", s=NS)
                m_sl = mask_sb[:, MOFF[h]:MOFF[h] + L]
                # partition ranges actually written by KQ matmuls (for L<32
                # the top of each 32-slot is unwritten PSUM -> skip it)
                pranges = [(0, 128)] if L >= 32 else \
                    [(k * 32, L) for k in range(g)]
                for S0 in range(0, NS, s_per):
                    ps_kq = kq_ps.tile([128, mg * L], F32, tag="kq")
                    for ci in range(mg):
                        for pi in range(g):
                            s = S0 + ci * g + pi
                            nc.tensor.matmul(
                                ps_kq[pi * sl:pi * sl + L,
                                      ci * L:(ci + 1) * L],
                                kt_r[:, :, s], qt_r[:, :, s],
                                start=True, stop=True,
                                tile_position=(0, pi * sl))
                    if sub < 2:
                        continue
                    numer = sm_pool.tile([128, mg * L], F32, tag="numer")
                    enumer = sm_pool.tile([128, mg * L], BF16, tag="enumer")
                    sums = small.tile([128, mg], F32, tag="sums")
                    recip = small.tile([128, mg], F32, tag="recip")
                    smkq = sm_pool.tile([128, mg * L], BF16, tag="smkq")
                    for p0, pl in pranges:
                        pz = slice(p0, p0 + pl)
                        mk = m_sl[pz, :]
                        mask_bc = bass.AP(tensor=mk.tensor, offset=mk.offset,
                                          ap=[mk.ap[0], [0, mg], mk.ap[1]])
                        nc.vector.tensor_add(numer[pz, :], ps_kq[pz, :],
                                             mask_bc)
                        nc.scalar.activation(
                            enumer[pz, :], numer[pz, :],
                            mybir.ActivationFunctionType.Exp, scale=NORM)
                        nc.vector.reduce_sum(
                            sums[pz, :],
                            enumer[pz, :].rearrange("p (c l) -> p c l", l=L),
                            axis=AX.X)
                        nc.vector.reciprocal(recip[pz, :], sums[pz, :])
                        rc = recip[pz, :]
                        rc_bc = bass.AP(tensor=rc.tensor, offset=rc.offset,
                                        ap=[rc.ap[0], [1, mg], [0, L]])
                        nc.vector.tensor_mul(smkq[pz, :], enumer[pz, :],
                                             rc_bc)
                    # att: one PSUM tile per partition slot — concurrent
                    # row-tiled matmuls must hit different PSUM banks
                    if sub < 3:
                        continue
                    cnt = s_per // g    # s values per slot in this S0 batch
                    for pi in range(g):
                        slot = pi * sl
                        ps_at = at_ps.tile([128, cnt * L], F32,
                                           tag=f"at{pi}", name=f"at{pi}",
                                           bufs=1)
                        for ci in range(cnt):
                            s = S0 + ci * g + pi
                            lhsT = vnat[h][slot:slot + L, s // g, :]
                            rhs = smkq.rearrange(
                                "p (c l) -> p c l", l=L)[slot:slot + L, ci, :]
                            nc.tensor.matmul(ps_at[:, ci * L:(ci + 1) * L],
                                             lhsT, rhs,
                                             start=True, stop=True,
                                             tile_position=(slot, 0))
                        # scatter-copy into dense atT[h] at cols s*64 + dil*l
                        if sub < 4:
                            continue
                        dil = DILS[h]
                        in_ap = ps_at.rearrange("p (s l) -> p s l", l=L)
                        s0 = S0 + pi
                        out_ap = atT[h].rearrange(
                            "p (s o) -> p s o", o=64
                        )[:, s0:s0 + (cnt - 1) * g + 1:g, 0:L * dil:dil] \
                            if dil > 1 else atT[h].rearrange(
                            "p (s o) -> p s o", o=64
                        )[:, s0:s0 + (cnt - 1) * g + 1:g, :]
                        nc.vector.tensor_copy(out=out_ap, in_=in_ap)

        # ---------------- phase E: output projection + store ---------------
        if max_phase < 5:
            return
        with ExitStack() as pctx:
            y_ps = pctx.enter_context(
                tc.tile_pool(name="y_ps", bufs=2, space="PSUM"))
            yo_pool = pctx.enter_context(tc.tile_pool(name="y_sb", bufs=3))
            for sp in range(NS // 2):
                ps_y = y_ps.tile([128, E], F32, tag="y")
                for h in range(H):
                    lhsT = atT[h][:, sp * 128:(sp + 1) * 128]
                    for half in range(2):
                        cs = half * 512
                        nc.tensor.matmul(
                            ps_y[:, cs:cs + 512], lhsT,
                            wout_sb[:, h * E + cs:h * E + cs + 512],
                            start=(h == 0), stop=(h == H - 1))
                y_sb = yo_pool.tile([128, E], F32)
                # copy + bias add in one DVE pass
                nc.vector.tensor_add(y_sb, ps_y, bias_sb)
                nc.sync.dma_start(out=y[sp * 128:(sp + 1) * 128, :],
                                  in_=y_sb)
    nc.finalize()
    return nc


_NC = None


def _get_program():
    global _NC
    if _NC is None:
        _NC = build_program()
    return _NC


def _host_inputs(Wk, Wq, Wv, W_out, b_out):
    bf = ml_dtypes.bfloat16
    Wstack = np.stack([Wq, Wk, Wv], 1)                     # [H, 3, 128, 1024]
    tmp = Wstack.reshape(H, 3, 128, NECHUNK, 128)          # [h, p, c, ec, r]
    wqkv_sb = np.ascontiguousarray(
        tmp.transpose(4, 0, 1, 3, 2)).reshape(128, -1).astype(bf)
    wout_sb = np.ascontiguousarray(
        W_out.reshape(E, H, 128).transpose(2, 1, 0)).reshape(128, H * E
                                                             ).astype(bf)
    mask_host = np.full((128, 120), NEG, np.float32)
    for h in range(H):
        L, sl = LS[h], SLOTL[h]
        for p in range(128):
            n = p % sl
            if n < L:
                mask_host[p, MOFF[h]:MOFF[h] + n + 1] = 0.0
            else:
                mask_host[p, MOFF[h]] = 0.0   # keep garbage rows finite
    bias_sb = np.ascontiguousarray(
        np.broadcast_to(np.asarray(b_out, np.float32).reshape(1, E),
                        (128, E)))
    return wqkv_sb, wout_sb, mask_host, bias_sb


def kernel(x, Wk, Wq, Wv, W_out, b_out):
    x = np.asarray(x, np.float32)
    wqkv_sb, wout_sb, mask_host, bias_sb = _host_inputs(
        np.asarray(Wk, np.float32), np.asarray(Wq, np.float32),
        np.asarray(Wv, np.float32), np.asarray(W_out, np.float32),
        np.asarray(b_out, np.float32))
    in_maps = []
    for c in range(8):
        b, half = c // 2, c % 2
        xs = np.ascontiguousarray(
            x[b].reshape(NB, SEG, E)[:, half * NS:(half + 1) * NS, :]
        ).reshape(ROWS, E)
        in_maps.append({"xs": xs, "wqkv": wqkv_sb, "wout": wout_sb,
                        "masks": mask_host, "bias": bias_sb})
    nc = _get_program()
    res = run_bass_kernel_spmd(nc, in_maps, core_ids=list(range(8)))
    y = np.empty((B, T, E), np.float32)
    for c in range(8):
        b, half = c // 2, c % 2
        y[b, half * ROWS:(half + 1) * ROWS, :] = res.results[c]["y"]
    return y
